# revision 10
# baseline (speedup 1.0000x reference)
"""Trainium2 Bass kernel for LoRALayer: out = 2.0 * (x @ B) @ A.

x: [4, 4096, 4096] f32; A: [8, 4096] f32; B: [4096, 8] f32.
Sharding: data-parallel on the 16384 tokens across 8 cores (2048 each);
A/B replicated. Host-side prep (part of sharding): each core's x-shard is
shipped transposed (contraction dim on SBUF partitions) as plain bf16;
B and 2*A likewise. All-bf16 numerics land at ~7e-3 absmax-rel vs the
f32 reference (f32 PSUM accumulation), inside the 2e-2 gate. Output is
written bf16 and upcast to f32 on the host.

The PE is the bottleneck on this part (power governor holds the PE near
1.2 GHz under sustained load): mm1 must ingest x at 256 B/cycle and mm2
must emit out at 128 elem/cycle -> 131072 PE cycles/core ~ 109 us.
DMA (33.6 MB/core at ~358 GB/s) is ~94 us and hides under it. So the
schedule aims to keep the PE stream dense: fine-grained first-block
input DMAs (256 KB) so mm1 starts ASAP, mm2 of block b-1 interleaved
1:1 with mm1 of block b, 2-bank PSUM output tiles with [128,1024]
PSUM->SBUF copies alternating DVE/ACT, and half-row (512 KB) output
DMAs issued as soon as each half of an output subtile is ready.

Per core, per 512-token block (f32 PSUM accumulation):
  mm1: ps_y[8,512] += B_c.T @ xt_c  over 32 feature chunks (K=128 each)
  y_sb = bf16(ps_y)  (DVE)
  mm2: per 128-token subtile, 8 chunks: o_ps[128,512] = y_sb.T @ A2[:,n]
       copy pairs [128,1024] f32->bf16, DMA halves [128,2048] bf16.
"""

import numpy as np

P = 128
F_IN = 4096
F_OUT = 4096
RANK = 8
N_CORES = 8
SCALING = 2.0
TBLK = 256             # token block (mm1 rhs free dim; short blocks = short tail)
CGRP0 = 4              # chunks per input sub-DMA, first block (256 KB, 2KB lines)
CGRP = 8               # chunks per input sub-DMA, later blocks (512 KB)

_CACHE = {}


def _build_nc(T, F_in, F_out, R):
    """Build the single-core Bass program for a T-token shard."""
    from contextlib import ExitStack

    import concourse.mybir as mybir
    import concourse.tile as tile
    from concourse import bacc

    f32 = mybir.dt.float32
    bf16 = mybir.dt.bfloat16
    tblk = min(TBLK, T)
    CH = F_in // P          # feature chunks (32)
    NB = T // tblk          # token blocks (4)
    NSUB = tblk // P        # 128-token subtiles per block (4)
    NS = F_out // 512       # output column chunks (8)
    NDMA0 = CH // CGRP0     # first-block input sub-DMAs (16)
    NDMA = CH // CGRP       # later-block input sub-DMAs (4)
    MM2_PER_BLK = NSUB * NS  # 32

    nc = bacc.Bacc("TRN2", target_bir_lowering=False, debug=False)

    xt0_d = nc.dram_tensor(
        "xt0", [NDMA0, P, CGRP0 * tblk], bf16, kind="ExternalInput"
    ).ap()
    xtr_d = nc.dram_tensor(
        "xtr", [max(NB - 1, 1), NDMA, P, CGRP * tblk], bf16, kind="ExternalInput"
    ).ap()
    bpk_d = nc.dram_tensor("Bpk", [P, CH * R], bf16, kind="ExternalInput").ap()
    a2_d = nc.dram_tensor("A2", [R, F_out], bf16, kind="ExternalInput").ap()
    out_d = nc.dram_tensor("out", [T, F_out], bf16, kind="ExternalOutput").ap()

    with tile.TileContext(nc) as tc, ExitStack() as ctx:
        cpool = ctx.enter_context(tc.tile_pool(name="const", bufs=1))
        xt0pool = ctx.enter_context(tc.tile_pool(name="xt0", bufs=NDMA0))
        xtpool = ctx.enter_context(tc.tile_pool(name="xt", bufs=2 * NDMA))
        ytpool = ctx.enter_context(tc.tile_pool(name="yt", bufs=2))
        opool = ctx.enter_context(tc.tile_pool(name="osb", bufs=6))
        y_pp = ctx.enter_context(tc.tile_pool(name="y_ps", bufs=1, space="PSUM"))
        o_pp = ctx.enter_context(tc.tile_pool(name="o_ps", bufs=7, space="PSUM"))

        bpk_sb = cpool.tile([P, CH * R], bf16, tag="bpk_sb")
        nc.sync.dma_start(bpk_sb[:], bpk_d)
        a2_sb = cpool.tile([R, F_out], bf16, tag="a2_sb")
        nc.sync.dma_start(a2_sb[:], a2_d)

        blk_state = {}

        def emit_mm2(blk, idx):
            """idx in [0, MM2_PER_BLK): (sub, n) pair for block blk."""
            sub, n = divmod(idx, NS)
            y_sb, o_sbs, o_pss = blk_state[blk]
            if n == 0:
                o_sbs[sub] = opool.tile(
                    [P, F_out], bf16, tag="o_sb", name=f"o_sb_{blk}_{sub}"
                )
            o_sb = o_sbs[sub]
            o_ps = o_pp.tile([P, 512], f32, tag="o_ps", name=f"o_ps_{blk}_{sub}_{n}")
            nc.tensor.matmul(
                o_ps[:],
                y_sb[:, sub * P:(sub + 1) * P],
                a2_sb[:, n * 512:(n + 1) * 512],
                start=True,
                stop=True,
            )
            dst = o_sb[:, n * 512:(n + 1) * 512]
            if n % 2 == 0:
                nc.vector.tensor_copy(dst, o_ps[:])
            else:
                nc.scalar.copy(dst, o_ps[:])
            trow = blk * tblk + sub * P
            if blk < NB - 1:
                # Mid-kernel: one full-row DMA per subtile (8 KB lines, best
                # queue efficiency; opool depth decouples PE from drain).
                if n == NS - 1:
                    nc.scalar.dma_start(out_d[trow:trow + P, :], o_sb[:])
            else:
                # Last block: quarter-row DMAs as soon as each pair of copies
                # lands, issued from the (now idle) sync ring so the ACT queue
                # keeps serving copies; overlaps the final drain with the
                # remaining matmuls.
                if n % 2 == 1:
                    cols = slice((n - 1) * 512, (n + 1) * 512)
                    nc.sync.dma_start(out_d[trow:trow + P, cols], o_sb[:, cols])

        for blk in range(NB + 1):
            xts = []
            cgrp = CGRP0 if blk == 0 else CGRP
            if blk < NB:
                for s in range(CH // cgrp):
                    pool = xt0pool if blk == 0 else xtpool
                    xt_sb = pool.tile(
                        [P, cgrp, tblk], bf16,
                        tag="xt0_sb" if blk == 0 else "xt_sb",
                    )
                    src = xt0_d[s] if blk == 0 else xtr_d[blk - 1, s]
                    nc.sync.dma_start(
                        xt_sb[:].rearrange("p c t -> p (c t)"), src
                    )
                    xts.append(xt_sb)
                ps_y = y_pp.tile([R, tblk], f32, tag="ps_y")

            # Spread mm2 of the previous block evenly among this block's mm1
            # chunks (mm2 kept slightly ahead: its copy chain has latency).
            a = CH if blk < NB else 0
            b = MM2_PER_BLK if blk > 0 else 0
            i = j = 0
            while i < a or j < b:
                if j < b and (i >= a or j * a <= i * b):
                    emit_mm2(blk - 1, j)
                    j += 1
                else:
                    c = i
                    nc.tensor.matmul(
                        ps_y[:],
                        bpk_sb[:, c * R:(c + 1) * R],
                        xts[c // cgrp][:, c % cgrp, :],
                        start=(c == 0),
                        stop=(c == CH - 1),
                    )
                    i += 1
            if blk > 0:
                del blk_state[blk - 1]
            if blk < NB:
                y_sb = ytpool.tile([R, tblk], bf16, tag="y_sb")
                nc.vector.tensor_copy(y_sb[:], ps_y[:])
                blk_state[blk] = (y_sb, {}, {})

    nc.compile()
    return nc


def _pack_inputs(x2d, A, B, T_shard, F_in, R):
    """Shard x on tokens (transposed, bf16); replicate bf16 B/2A packs."""
    import ml_dtypes

    bf16 = ml_dtypes.bfloat16
    CH = F_in // P

    # chunk-major B pack: col block c holds B chunk c ([128, R])
    bpk = np.ascontiguousarray(
        B.astype(np.float32).astype(bf16).reshape(CH, P, R)
        .transpose(1, 0, 2).reshape(P, CH * R)
    )
    a2 = np.ascontiguousarray((SCALING * A).astype(np.float32).astype(bf16))

    # device-DMA-friendly packs: per sub-DMA one contiguous per-partition run.
    T = T_shard
    tblk = min(TBLK, T)
    NB = T // tblk

    def pack(m, nb, cgrp):
        ndma = CH // cgrp
        a = m.reshape(ndma, cgrp, P, nb, tblk)
        a = a.transpose(3, 0, 2, 1, 4)
        return np.ascontiguousarray(a.reshape(nb, ndma, P, cgrp * tblk))

    n_shards = x2d.shape[0] // T_shard
    in_maps = []
    for c in range(n_shards):
        xt = x2d[c * T_shard:(c + 1) * T_shard].T.astype(bf16)
        xt3 = xt.reshape(F_in, NB, tblk)
        xt0 = pack(np.ascontiguousarray(xt3[:, 0]), 1, CGRP0)[0]
        if NB > 1:
            xtr = pack(
                np.ascontiguousarray(
                    xt3[:, 1:].transpose(0, 1, 2).reshape(F_in, (NB - 1) * tblk)
                ),
                NB - 1, CGRP,
            )
        else:
            xtr = np.zeros((1, CH // CGRP, P, CGRP * tblk), dtype=bf16)
        in_maps.append({"xt0": xt0, "xtr": xtr, "Bpk": bpk, "A2": a2})
    return in_maps


def kernel(x, A, B):
    from concourse.bass_utils import run_bass_kernel_spmd

    x = np.asarray(x, dtype=np.float32)
    A = np.asarray(A, dtype=np.float32)
    B = np.asarray(B, dtype=np.float32)
    orig_shape = x.shape
    x2d = x.reshape(-1, F_IN)
    T_shard = x2d.shape[0] // N_CORES

    key = (T_shard, F_IN, F_OUT, RANK)
    if key not in _CACHE:
        _CACHE[key] = _build_nc(T_shard, F_IN, F_OUT, RANK)
    nc = _CACHE[key]

    in_maps = _pack_inputs(x2d, A, B, T_shard, F_IN, RANK)
    res = run_bass_kernel_spmd(nc, in_maps, core_ids=list(range(N_CORES)))
    out = np.concatenate(
        [np.asarray(r["out"], dtype=np.float32) for r in res.results], axis=0
    )
    return out.reshape(*orig_shape[:-1], F_OUT)
